# revision 44
# baseline (speedup 1.0000x reference)
"""Multi-head attention kernel for Trainium2, 8 NeuronCores (v2).

Problem (hardcoded): B=4, S=2048, E=1024, H=16, DH=64.
  q/k/v = einsum('bse,hed->bhsd', x, W{q,k,v}) + b{q,k,v}
  attn  = softmax(q k^T / sqrt(DH)) v ;  out = concat(attn) @ Wo^T + bo

Sharding: core c -> (batch c//2, head-half c%2: 8 heads, 512 concat cols).
Host sums the two partial out-projections per batch and adds
bo_eff = bo + Wo @ bv_flat (the v-bias commutes through softmax-weighted
averaging, so it is folded into the output bias on the host).

Cost-model-driven design (PE charge = out-free rows x cycles/row; fp8
DoubleRow = 0.5, stationary operand loads are free):
  - projections: 3-term fp8 DoubleRow over a stacked contraction
    (x8*w8 + xr8*w8 + x8*wr8), K=3072 in 12 DR chunks       ~147K cyc
  - scores: 4-term pair-fp8 DoubleRow, K=256=(q8|qr8)x(k8|kr8)
    in ONE DR matmul per t-block: bf16-grade accuracy at
    half bf16 cost                                          ~131K cyc
  - attnV: out [s,65] with exp-scores as the stationary op   ~133K cyc
  - exp: ACT true-exp for 6/8 groups; DVE Schraudolph
    (i16 = rint(s*0.125*128/ln2 + 16256-7.37) bitcast bf16) for 2/8
  - normalize: gpsimd normalize_recip off the ones-column sums
  - concat transposes on PE (bf16) + DVE 2x copies; bf16 out-proj
"""

import os
import sys

for _p in ("/opt/trn_rl_repo", "/root/.axon_site/_ro/trn_rl_repo"):
    if os.path.isdir(_p) and _p not in sys.path:
        sys.path.insert(0, _p)
        break

from collections import deque
from contextlib import ExitStack

import numpy as np
import ml_dtypes

import concourse.bass as bass
import concourse.tile as tile
import concourse.mybir as mybir
from concourse import bacc, bass_utils

B, S, E, H, DH = 4, 2048, 1024, 16, 64
HPC = 8             # heads per core
JW = HPC * DH       # 512
SB = S // 128       # 16 s/t-blocks
EB = E // 128       # 8 e-blocks
SC = S // 512       # 4 s-chunks
NJB = JW // 128     # 4 j-blocks
N_CORES = 8

F32 = mybir.dt.float32
BF16 = mybir.dt.bfloat16
FP8 = mybir.dt.float8e4
I16 = mybir.dt.int16
Exp = mybir.ActivationFunctionType.Exp
ADD = mybir.AluOpType.add
SUB = mybir.AluOpType.subtract
MULT = mybir.AluOpType.mult
DR = mybir.MatmulPerfMode.DoubleRow

NP8 = ml_dtypes.float8_e4m3
NPBF = ml_dtypes.bfloat16

# fp8 e4m3 loses precision near its subnormal range (min normal 2^-6), so
# operands are pre-scaled into mid-range before pair-quantization and the
# descales are folded into existing scalar ops:
XSC = 8.0      # x scaled by 8 on host
WSC = 64.0     # Wq/Wk/Wv scaled by 64 on host
QSC = 16.0     # q/k staged at 16x in bf16/fp8
PSUM_TO_QBF = QSC / (XSC * WSC)        # proj psum -> 16x(q+bias)
PSUM_TO_V = 1.0 / (XSC * WSC)          # proj psum -> v
SCORE_SCALE = 0.125 / (QSC * QSC)      # scores psum -> exp argument

# schraudolph: i16 = rint(arg * 128/ln2 + (127*128 - 7.37))
SCH_C1 = float(SCORE_SCALE * 128.0 / np.log(2.0))
SCH_C2 = float(127.0 * 128.0 - 7.37)
# which of the 8 exp groups per (h, sc) go to DVE via schraudolph
SCH_PAT = (False, True, False, False, True, False, True, False)


def _emit(tc, aps, ctx, dbg=None):
    nc = tc.nc
    (x8_d, xr8_d, wq_d, wk_d, wv_d, wo_d, bqc_d, bkc_d, id_d, out_d) = aps

    def pool(**kw):
        return ctx.enter_context(tc.tile_pool(**kw))

    const = pool(name="const", bufs=1)
    xp = pool(name="xp", bufs=1)
    qbfp = pool(name="qbf", bufs=2)
    q8p = pool(name="q8p", bufs=2)
    qkT = pool(name="qkT", bufs=4)
    vxp = pool(name="vext", bufs=1)
    exp_p = pool(name="expS", bufs=2)
    anp = pool(name="attn_n", bufs=8)
    asbp = pool(name="acc_sb", bufs=2)
    ccp = pool(name="concatT", bufs=1)
    outp = pool(name="outs", bufs=2)
    ps_sc = pool(name="ps_sc", bufs=3, space="PSUM")   # scores [128,1024]
    ps_ac = pool(name="ps_ac", bufs=1, space="PSUM")   # attn acc [128,512]
    ps_pj = pool(name="ps_pj", bufs=1, space="PSUM")   # proj/outproj [128,512]

    # ---- constants / weights ----
    ident = const.tile([128, 128], BF16)
    nc.sync.dma_start(ident[:], id_d[:])
    bqc = const.tile([128, NJB], F32)
    nc.sync.dma_start(bqc[:], bqc_d[:])
    bkc = const.tile([128, NJB], F32)
    nc.sync.dma_start(bkc[:], bkc_d[:])
    wq_sb = const.tile([128, 2, EB, JW], FP8)
    wk_sb = const.tile([128, 2, EB, JW], FP8)
    wv_sb = const.tile([128, 2, EB, JW], FP8)
    wo_sb = const.tile([128, 4, E], BF16)

    def load_wqk():
        nc.sync.dma_start(wq_sb[:], wq_d.rearrange("(eb p) w j -> p w eb j", p=128))
        nc.sync.dma_start(wk_sb[:], wk_d.rearrange("(eb p) w j -> p w eb j", p=128))

    def load_wv():
        nc.sync.dma_start(wv_sb[:], wv_d.rearrange("(eb p) w j -> p w eb j", p=128))

    def load_wo():
        nc.sync.dma_start(wo_sb[:], wo_d.rearrange("(fb p) e -> p fb e", p=128))

    x8 = xp.tile([128, EB, S], FP8)
    xr8 = xp.tile([128, EB, S], FP8)

    def dma_x(sc):
        # x8 on the SP queue, xr8 on the (prefix-idle) ACT queue in parallel
        sl = slice(sc * 512, (sc + 1) * 512)
        nc.sync.dma_start(
            x8[:, :, sl], x8_d.rearrange("(eb p) s -> p eb s", p=128)[:, :, sl])
        nc.scalar.dma_start(
            xr8[:, :, sl], xr8_d.rearrange("(eb p) s -> p eb s", p=128)[:, :, sl])

    vext = vxp.tile([128, SB, HPC, DH + 1], BF16)

    # 3-term DR chunk list: (x operand, w selector)
    TERMS = ((x8, 0), (xr8, 0), (x8, 1))

    def emit_v(tb):
        """v for all 8 heads of t-block tb -> vext[:, tb] (no bias)."""
        pv = ps_pj.tile([128, 512], F32, tag="pj", name=f"pv{tb}")
        nmm = 3 * (EB // 2)
        k = 0
        for xt, ws in TERMS:
            for i in range(EB // 2):
                nc.tensor.matmul(
                    pv[:], xt[:, 2 * i:2 * i + 2, tb * 128:(tb + 1) * 128],
                    wv_sb[:, ws, 2 * i:2 * i + 2, :],
                    start=(k == 0), stop=(k == nmm - 1), perf_mode=DR)
                k += 1
        nc.vector.tensor_scalar(
            vext[:, tb, :, 0:DH],
            pv[:].rearrange("p (h d) -> p h d", h=HPC),
            PSUM_TO_V, None, MULT)

    def emit_qk_chunk(jb, sc, which, q_bf, k_bf, q8, qr8, k8, kr8):
        """One (jb, sc) projection of q or k: 12 DR matmuls + bias copy +
        fp8 pair conversion of this s-chunk (Pool)."""
        w_sb, bc, dst, d8, dr8 = ((wq_sb, bqc, q_bf, q8, qr8) if which == "q"
                                  else (wk_sb, bkc, k_bf, k8, kr8))
        pq = ps_pj.tile([128, 512], F32, tag="pj", name=f"p{which}{jb}_{sc}")
        nmm = 3 * (EB // 2)
        k = 0
        for xt, ws in TERMS:
            for i in range(EB // 2):
                nc.tensor.matmul(
                    pq[:], w_sb[:, ws, 2 * i:2 * i + 2, jb * 128:(jb + 1) * 128],
                    xt[:, 2 * i:2 * i + 2, sc * 512:(sc + 1) * 512],
                    start=(k == 0), stop=(k == nmm - 1), perf_mode=DR)
                k += 1
        sl = slice(sc * 512, (sc + 1) * 512)
        nc.vector.tensor_scalar(
            dst[:, sl], pq[:], PSUM_TO_QBF, bc[:, jb:jb + 1], MULT, ADD)
        nc.gpsimd.tensor_copy(d8[:, sl], dst[:, sl])
        nc.gpsimd.tensor_tensor(dr8[:, sl], dst[:, sl], d8[:, sl], SUB)

    def emit_jb(jb):
        """PE closures for j-block jb: projections (+ per-chunk converts),
        then rearrange DMAs producing qT/kT tiles for heads 2jb, 2jb+1."""
        q_bf = qbfp.tile([128, S], BF16, tag="qbf", name=f"qbf{jb}")
        k_bf = qbfp.tile([128, S], BF16, tag="kbf", name=f"kbf{jb}")
        q8 = q8p.tile([128, S], FP8, tag="q8", name=f"q8_{jb}")
        qr8 = q8p.tile([128, S], FP8, tag="qr8", name=f"qr8_{jb}")
        k8 = q8p.tile([128, S], FP8, tag="k8", name=f"k8_{jb}")
        kr8 = q8p.tile([128, S], FP8, tag="kr8", name=f"kr8_{jb}")
        chunks = []
        for sc in range(SC):
            chunks.append(lambda sc=sc: emit_qk_chunk(
                jb, sc, "q", q_bf, k_bf, q8, qr8, k8, kr8))
        for sc in range(SC):
            chunks.append(lambda sc=sc: emit_qk_chunk(
                jb, sc, "k", q_bf, k_bf, q8, qr8, k8, kr8))

        def finish():
            for hl in range(2):
                h = 2 * jb + hl
                hoff = hl * 64
                qT = qkT.tile([128, 2, S], FP8, tag="qT", name=f"qT{h}")
                kT = qkT.tile([128, 2, S], FP8, tag="kT", name=f"kT{h}")
                for kt in range(2):
                    nc.sync.dma_start(qT[0:64, kt, :], q8[hoff:hoff + 64, :])
                    nc.sync.dma_start(qT[64:128, kt, :], qr8[hoff:hoff + 64, :])
                for half in range(2):
                    hsl = slice(half * 64, (half + 1) * 64)
                    nc.sync.dma_start(kT[hsl, 0, :], k8[hoff:hoff + 64, :])
                    nc.sync.dma_start(kT[hsl, 1, :], kr8[hoff:hoff + 64, :])
                qkT_tiles[h] = (qT, kT)
            if dbg is not None and jb == 0:
                nc.sync.dma_start(dbg["qbf0"][:], q_bf[:])
                nc.sync.dma_start(dbg["q8_0"][:], q8[:])
                qT0, kT0 = qkT_tiles[0]
                nc.sync.dma_start(
                    dbg["qT0"][:], qT0[:].rearrange("p a b -> p (a b)"))
                nc.sync.dma_start(
                    dbg["kT0"][:], kT0[:].rearrange("p a b -> p (a b)"))

        chunks.append(lambda: finish())
        return chunks

    qkT_tiles = {}
    concatT = ccp.tile([128, 4, S], BF16)
    attn_n = {}
    pe_feed = deque()

    def drain(n):
        for _ in range(n):
            if pe_feed:
                pe_feed.popleft()()

    def emit_outproj(sc, sblk, ec):
        def emit():
            off = sc * 512 + sblk * 128
            po = ps_sc.tile([128, 1024], F32, tag="sc",
                            name=f"po{sc}_{sblk}_{ec}")[:, 0:512]
            for fb in range(4):
                nc.tensor.matmul(po[:], concatT[:, fb, off:off + 128],
                                 wo_sb[:, fb, ec * 512:(ec + 1) * 512],
                                 start=(fb == 0), stop=(fb == 3))
            ot = outp.tile([128, 512], F32, tag="ot", name=f"ot{sc}_{sblk}_{ec}")
            nc.vector.tensor_copy(ot[:], po[:])
            nc.sync.dma_start(
                out_d[off:off + 128, ec * 512:(ec + 1) * 512], ot[:])
        return emit

    # ---- attention stream with cross-iteration attnV lag (depth 2: the PE
    # is in-order, so each attnV must trail its exp by enough emitted work
    # that the exp has finished by the time the PE reaches the attnV) ----
    pend = deque()   # (h, sc, g, expS tile, acc tile)

    def flush_pend():
        if not pend:
            return
        h, sc, g, eS, acc = pend.popleft()
        # NOTE: a start=True matmul zeroes the whole PSUM bank, so only the
        # very first matmul into this tile may carry it; the other three
        # sblk regions accumulate with start=False onto the zeroed bank.
        for sblk in range(4):
            for t2 in range(2):
                tb = 2 * g + t2
                nc.tensor.matmul(
                    acc[:, sblk * 65:sblk * 65 + 65],
                    eS[:, tb, sblk * 128:(sblk + 1) * 128],
                    vext[:, tb, h, :],
                    start=(tb == 0 and sblk == 0), stop=(tb == SB - 1),
                    skip_group_check=True)
        if g == 7:
            finish_iter(h, sc, acc)

    def finish_iter(h, sc, acc):
        acc_sb = asbp.tile([128, 260], F32, tag="asb", name=f"asb{h}_{sc}")
        nc.vector.tensor_copy(acc_sb[:], acc[:, 0:260])
        if dbg is not None and h == 0 and sc == 0:
            nc.sync.dma_start(dbg["asb00"][:], acc_sb[:])
            nc.sync.dma_start(
                dbg["vext"][:], vext[:].rearrange("p a b c -> p (a b c)"))
        if h % 2 == 0:
            attn_n[sc] = anp.tile([128, 4, 128], BF16, tag="an",
                                  name=f"an{h}_{sc}")
        an = attn_n[sc]
        hc = (h % 2) * 64
        for sblk in range(4):
            nc.gpsimd.normalize_recip(
                an[:, sblk, hc:hc + 64],
                acc_sb[:, sblk * 65:sblk * 65 + 64],
                acc_sb[:, sblk * 65 + 64:sblk * 65 + 65])
        if h % 2 == 1:
            # SBUF->SBUF XBAR transpose straight into concatT
            for sblk in range(4):
                nc.sync.dma_start_transpose(
                    concatT[:, h // 2, sc * 512 + sblk * 128:
                            sc * 512 + (sblk + 1) * 128],
                    an[:, sblk, :])
            if h == HPC - 1:
                for sblk in range(4):
                    for ec in range(2):
                        pe_feed.append(emit_outproj(sc, sblk, ec))
            if dbg is not None and h == 1 and sc == 0:
                nc.sync.dma_start(dbg["an0"][:],
                                  an[:].rearrange("p a b -> p (a b)"))

    def attn_iter(h, sc):
        qT, kT = qkT_tiles[h]
        eS = exp_p.tile([128, SB, 512], BF16, tag="eS", name=f"eS{h}_{sc}")
        acc = ps_ac.tile([128, 512], F32, tag="ac", name=f"ac{h}_{sc}")
        for g in range(8):
            scp = ps_sc.tile([128, 1024], F32, tag="sc", name=f"s{h}_{sc}_{g}")
            for t2 in range(2):
                tb = 2 * g + t2
                nc.tensor.matmul(scp[:, t2 * 512:(t2 + 1) * 512],
                                 kT[:, :, tb * 128:(tb + 1) * 128],
                                 qT[:, :, sc * 512:(sc + 1) * 512],
                                 start=True, stop=True, perf_mode=DR)
            dst = eS[:, 2 * g:2 * g + 2, :].rearrange("p a b -> p (a b)")
            if SCH_PAT[g]:
                nc.vector.tensor_scalar(dst.bitcast(I16), scp[:],
                                        SCH_C1, SCH_C2, MULT, ADD)
            else:
                nc.scalar.activation(dst, scp[:], Exp, scale=SCORE_SCALE)
            if len(pend) >= 4:
                flush_pend()
            pend.append((h, sc, g, eS, acc))
            drain(2 if ((h == 0 and sc == 0) or h >= HPC - 2) else 1)
        if dbg is not None and h == 0 and sc == 0:
            nc.sync.dma_start(
                dbg["eS00"][:], eS[:].rearrange("p a b -> p (a b)"))

    # ---- prefix: q/k j-block 0 first (longest latency chain to the first
    # attention group), early v blocks next, the rest feeds the main loop ----
    nc.gpsimd.memset(vext[:, :, :, DH:DH + 1], 1.0)
    load_wqk()
    for sc in range(SC):
        dma_x(sc)
    for ch in emit_jb(0):
        ch()
    nc.scalar.dma_start(wv_sb[:], wv_d.rearrange("(eb p) w j -> p w eb j", p=128))
    nc.scalar.dma_start(wo_sb[:], wo_d.rearrange("(fb p) e -> p fb e", p=128))
    for tb in range(4):
        emit_v(tb)
    for tb in range(4, SB):
        pe_feed.append(lambda tb=tb: emit_v(tb))
    for jb in range(1, NJB):
        pe_feed.extend(emit_jb(jb))

    # ---- main loop ----
    # last head-pair interleaved by s-chunk so each sc's output projection
    # unlocks as early as possible instead of all piling up at the tail
    iters = [(h, sc) for h in range(HPC - 2) for sc in range(SC)]
    iters += [(h, sc) for sc in range(SC) for h in (HPC - 2, HPC - 1)]
    for h, sc in iters:
        attn_iter(h, sc)
    while pend:
        flush_pend()
    drain(len(pe_feed))
    if dbg is not None:
        nc.sync.dma_start(dbg["cc"][:],
                          concatT[:].rearrange("p a b -> p (a b)"))


_CACHE = {}


def _build():
    nc = bacc.Bacc("TRN2", target_bir_lowering=False, debug=False,
                   num_devices=N_CORES)
    x8_d = nc.dram_tensor("x8", [E, S], FP8, kind="ExternalInput").ap()
    xr8_d = nc.dram_tensor("xr8", [E, S], FP8, kind="ExternalInput").ap()
    wq_d = nc.dram_tensor("wq", [E, 2, JW], FP8, kind="ExternalInput").ap()
    wk_d = nc.dram_tensor("wk", [E, 2, JW], FP8, kind="ExternalInput").ap()
    wv_d = nc.dram_tensor("wv", [E, 2, JW], FP8, kind="ExternalInput").ap()
    wo_d = nc.dram_tensor("wo", [JW, E], BF16, kind="ExternalInput").ap()
    bqc_d = nc.dram_tensor("bqc", [128, NJB], F32, kind="ExternalInput").ap()
    bkc_d = nc.dram_tensor("bkc", [128, NJB], F32, kind="ExternalInput").ap()
    id_d = nc.dram_tensor("ident", [128, 128], BF16, kind="ExternalInput").ap()
    out_d = nc.dram_tensor("out", [S, E], F32, kind="ExternalOutput").ap()
    aps = (x8_d, xr8_d, wq_d, wk_d, wv_d, wo_d, bqc_d, bkc_d, id_d, out_d)
    with tile.TileContext(nc) as tc:
        with ExitStack() as ctx:
            _emit(tc, aps, ctx)
    nc.compile()
    return nc


def _prep_core(x, Wq, bq, Wk, bk, Wv, bv, Wo, c):
    b, hh = c // 2, c % 2
    hs = slice(hh * HPC, (hh + 1) * HPC)
    xT = np.ascontiguousarray(x[b].T) * np.float32(XSC)     # [E, S]
    x8 = xT.astype(NP8)
    xr8 = (xT - x8.astype(np.float32)).astype(NP8)

    def wpair(W):
        Wc = np.ascontiguousarray(
            W[hs].transpose(1, 0, 2).reshape(E, JW)) * np.float32(WSC)
        w8 = Wc.astype(NP8)
        wr8 = (Wc - w8.astype(np.float32)).astype(NP8)
        return np.ascontiguousarray(np.stack([w8, wr8], axis=1))  # [E, 2, JW]

    WoT = np.ascontiguousarray(Wo.T)                        # [f, e]
    return {
        "x8": x8, "xr8": xr8,
        "wq": wpair(Wq), "wk": wpair(Wk), "wv": wpair(Wv),
        "wo": np.ascontiguousarray(
            WoT[hh * JW:(hh + 1) * JW]).astype(NPBF),
        "bqc": np.ascontiguousarray(
            bq[hs].reshape(NJB, 128).T * np.float32(QSC)),
        "bkc": np.ascontiguousarray(
            bk[hs].reshape(NJB, 128).T * np.float32(QSC)),
        "ident": np.eye(128, dtype=np.float32).astype(NPBF),
    }


def kernel(x, Wq, bq, Wk, bk, Wv, bv, Wo, bo):
    x = np.asarray(x, dtype=np.float32)
    Wq = np.asarray(Wq, dtype=np.float32)
    bq = np.asarray(bq, dtype=np.float32)
    Wk = np.asarray(Wk, dtype=np.float32)
    bk = np.asarray(bk, dtype=np.float32)
    Wv = np.asarray(Wv, dtype=np.float32)
    bv = np.asarray(bv, dtype=np.float32)
    Wo = np.asarray(Wo, dtype=np.float32)
    bo = np.asarray(bo, dtype=np.float32)

    if "nc" not in _CACHE:
        _CACHE["nc"] = _build()
    nc = _CACHE["nc"]

    in_maps = [_prep_core(x, Wq, bq, Wk, bk, Wv, bv, Wo, c)
               for c in range(N_CORES)]
    res = bass_utils.run_bass_kernel_spmd(nc, in_maps,
                                          core_ids=list(range(N_CORES)))
    bo_eff = bo + Wo @ bv.reshape(-1)
    out = np.empty((B, S, E), dtype=np.float32)
    for b in range(B):
        out[b] = res.results[2 * b]["out"] + res.results[2 * b + 1]["out"]
        out[b] += bo_eff[None, :]
    return out
